# revision 12
# baseline (speedup 1.0000x reference)
"""Multi-head attention (B=2, N=2048, D=1024, H=16, HD=64) on 8 trn2 NeuronCores.

Sharding: data-parallel over batch (2) x tensor-parallel over head groups (4).
Core c handles batch b=c//4, heads 4*(c%4)..4*(c%4)+3. Each core computes
Q/K/V projections for its head slice, attention, and a partial output
projection (its heads' rows of Wo); the host sums the 4 partials per batch
and adds bo.

Device layout strategy: everything lives feature-on-partitions ("transposed")
so no on-device transposes are needed:
  - host passes X[b].T; Q^T/K^T computed as (W^T X^T) with W as stationary.
  - V computed in native [token, d] layout (X^T tiles as stationary).
  - scores computed as S^T[j, i] (key j on partitions) so the mask bias is a
    per-partition scalar and softmax normalization is deferred:
    E^T = exp(S/8 + maskbias) via one ScalarE activation (PSUM->SBUF).
  - ctx^T[d, i] = sum_j V_aug[j, d] E^T[j, i]; V_aug has a ones column so the
    softmax denominator rides along as ctx row 64.
  - normalization multiplies ctx^T by 1/denom broadcast via a tiny PE matmul.
  - out^T = Wo^T ctx^T accumulated over head pairs; host transposes back.
"""

import sys

if "/opt/trn_rl_repo" not in sys.path:
    sys.path.insert(0, "/opt/trn_rl_repo")

import numpy as np

import concourse.bacc as bacc
import concourse.mybir as mybir
import concourse.tile as tile

B, N, D = 2, 2048, 1024
H, HD = 16, 64
HG = 4  # head groups (tensor parallel)
HPG = H // HG  # heads per group = 4
DG = HPG * HD  # feature slice per group = 256

F32 = mybir.dt.float32
# Matmul datapath dtype: bf16 streams 1 row/cycle at any size, halves HBM
# and SBUF traffic, and doubles DVE write throughput. Accumulation stays
# fp32 in PSUM. float32r is kept for the tiny norm-broadcast/bias matmuls
# whose operands must stay full precision.
MMT = mybir.dt.bfloat16
F32R = mybir.dt.float32r


def _mm_ap(ap):
    return ap


def build_program(loop_iters: int = 1):
    nc = bacc.Bacc("TRN2", target_bir_lowering=False)

    xt = nc.dram_tensor("xt", [D, N], MMT, kind="ExternalInput")
    wq = nc.dram_tensor("wq", [128, 8, DG], MMT, kind="ExternalInput")
    wk = nc.dram_tensor("wk", [128, 8, DG], MMT, kind="ExternalInput")
    wv = nc.dram_tensor("wv", [128, 8, DG], MMT, kind="ExternalInput")
    bq = nc.dram_tensor("bq", [128, 2], F32, kind="ExternalInput")
    bk = nc.dram_tensor("bk", [128, 2], F32, kind="ExternalInput")
    bvr = nc.dram_tensor("bvr", [1, DG], F32R, kind="ExternalInput")
    wo = nc.dram_tensor("wo", [128, 2, D], MMT, kind="ExternalInput")
    mb = nc.dram_tensor("mb", [128, 16], F32, kind="ExternalInput")
    onesin = nc.dram_tensor("onesin", [128, 128], F32R, kind="ExternalInput")
    onesb = nc.dram_tensor("onesb", [128, 64], MMT, kind="ExternalInput")
    outp = nc.dram_tensor("outp", [D, N], F32, kind="ExternalOutput")

    with tile.TileContext(nc) as tc, nc.allow_low_precision(
        reason="fp32r matmul datapath; accumulation stays fp32 in PSUM"
    ):
        import contextlib

        ctx = contextlib.ExitStack()
        with ctx:
            const = ctx.enter_context(tc.tile_pool(name="const", bufs=1))
            big = ctx.enter_context(tc.tile_pool(name="big", bufs=5))
            xtcp = ctx.enter_context(tc.tile_pool(name="xtcp", bufs=4))
            qk = ctx.enter_context(tc.tile_pool(name="qk", bufs=1))
            epool = ctx.enter_context(tc.tile_pool(name="epool", bufs=5))
            rpool = ctx.enter_context(tc.tile_pool(name="rpool", bufs=2))
            psum_b = ctx.enter_context(
                tc.tile_pool(name="psum_b", bufs=2, space="PSUM")
            )
            psum_c = ctx.enter_context(
                tc.tile_pool(name="psum_c", bufs=2, space="PSUM")
            )

            loop_cm = (
                tc.For_i(0, loop_iters, 1)
                if loop_iters > 1
                else contextlib.nullcontext()
            )
            with loop_cm:
                # ---- phase 1 loads first: wq + X chunk 0 gate the first
                # matmul chain, so issue them before the small const DMAs.
                wq_sb = big.tile([128, 8, DG], MMT, tag="big")
                nc.sync.dma_start(out=wq_sb[:, :, 0:128], in_=wq[:, :, 0:128])
                nc.sync.dma_start(out=wq_sb[:, :, 128:DG], in_=wq[:, :, 128:DG])
                wk_sb = big.tile([128, 8, DG], MMT, tag="big")
                nc.gpsimd.dma_start(out=wk_sb, in_=wk[:, :, :])
                wv_sb = big.tile([128, 8, DG], MMT, tag="big")
                nc.gpsimd.dma_start(out=wv_sb, in_=wv[:, :, :])

                # ---- constants ----
                ones = const.tile([128, 128], F32R, tag="ones")
                nc.sync.dma_start(out=ones, in_=onesin[:, :])
                bq_sb = const.tile([128, 2], F32, tag="bq")
                nc.sync.dma_start(out=bq_sb, in_=bq[:, :])
                bk_sb = const.tile([128, 2], F32, tag="bk")
                nc.sync.dma_start(out=bk_sb, in_=bk[:, :])
                bvr_sb = const.tile([1, DG], F32R, tag="bvr")
                nc.sync.dma_start(out=bvr_sb, in_=bvr[:, :])
                mb_sb = const.tile([128, 16], F32, tag="mb")
                nc.sync.dma_start(out=mb_sb, in_=mb[:, :])
                wo_sb = const.tile([128, 2, D], MMT, tag="wo")

                # bv broadcast to all 128 partitions via PE
                bv_ps = psum_b.tile([128, DG], F32, tag="bank")
                nc.tensor.matmul(
                    bv_ps, _mm_ap(ones[0:1, 0:128]), _mm_ap(bvr_sb[0:1, :]),
                    start=True, stop=True,
                )
                bv_bc = const.tile([128, DG], F32, tag="bvbc")
                nc.vector.tensor_copy(bv_bc, bv_ps)
                # X^T in 4 column-chunk tiles [128, kt=8, 512]; one DMA each,
                # chunk-major so early Q/K/V chains start before the full X
                # load completes. xtc[c][kt] view = features kt*128.., tokens
                # c*512..
                xtc_t = []
                for c in range(4):
                    t = xtcp.tile([128, 8, 512], MMT, tag="xtc", name="xtc")
                    eng = nc.sync if c < 2 else nc.gpsimd
                    nparts = 4 if c == 0 else 2
                    for h in range(nparts):
                        kpp = 8 // nparts
                        eng.dma_start(
                            out=t[:, h * kpp : (h + 1) * kpp, :],
                            in_=xt[
                                h * kpp * 128 : (h + 1) * kpp * 128,
                                c * 512 : (c + 1) * 512,
                            ].rearrange("(kt p) col -> p kt col", p=128),
                        )
                    xtc_t.append(t)
                xtc = [[xtc_t[c][:, kt, :] for kt in range(8)] for c in range(4)]

                qt_sb = [qk.tile([128, N], MMT, tag=f"qt{m}", name=f"qt{m}") for m in range(2)]
                kt_sb = [qk.tile([128, N], MMT, tag=f"kt{m}", name=f"kt{m}") for m in range(2)]
                # V with ones column appended per head: [128, jt, head, 65]
                v_sb = qk.tile([128, 16, HPG, HD + 1], MMT, tag="v")
                nc.gpsimd.dma_start(
                    out=v_sb[:, :, :, HD : HD + 1], in_=onesb[:, :]
                )

                def qk_chain(proj, mt, nt):
                    w_sb, bias_sb, dst = (
                        (wq_sb, bq_sb, qt_sb) if proj == 0 else (wk_sb, bk_sb, kt_sb)
                    )
                    ps = psum_b.tile([128, 512], F32, tag="bank", name="qkps")
                    for kt in range(8):
                        nc.tensor.matmul(
                            ps,
                            _mm_ap(w_sb[:, kt, mt * 128 : (mt + 1) * 128]),
                            _mm_ap(xtc[nt][kt]),
                            start=(kt == 0),
                            stop=(kt == 7),
                        )
                    nc.vector.tensor_scalar_add(
                        dst[mt][:, nt * 512 : (nt + 1) * 512],
                        ps,
                        bias_sb[:, mt : mt + 1],
                    )

                def v_chain(mt):
                    ps = psum_b.tile([128, DG], F32, tag="bank", name="vps")
                    for kt in range(8):
                        nc.tensor.matmul(
                            ps,
                            _mm_ap(
                                xtc[mt // 4][kt][
                                    :, (mt % 4) * 128 : (mt % 4 + 1) * 128
                                ]
                            ),
                            _mm_ap(wv_sb[:, kt, :]),
                            start=(kt == 0),
                            stop=(kt == 7),
                        )
                    nc.vector.tensor_tensor(
                        out=v_sb[:, mt, :, 0:HD],
                        in0=ps.rearrange("p (h d) -> p h d", h=HPG),
                        in1=bv_bc.rearrange("p (h d) -> p h d", h=HPG),
                        op=mybir.AluOpType.add,
                    )

                # chains needed before the unit stream starts
                for fn in (
                    lambda: qk_chain(0, 0, 0),
                    lambda: qk_chain(1, 0, 0),
                    lambda: qk_chain(0, 0, 1),
                    lambda: qk_chain(1, 0, 1),
                    lambda: v_chain(0),
                ):
                    fn()

                # remaining chains, spread through the unit stream (key =
                # iteration index at whose END the chain is emitted; each must
                # precede its first consumer unit)
                inserts = {}
                for i in range(1, 16):
                    inserts.setdefault(i, []).append(lambda m=i: v_chain(m))
                inserts.setdefault(6, []).append(lambda: qk_chain(1, 0, 2))
                inserts.setdefault(8, []).append(
                    lambda: nc.sync.dma_start(out=wo_sb, in_=wo[:, :, :])
                )
                inserts.setdefault(10, []).append(lambda: qk_chain(1, 0, 3))
                inserts.setdefault(11, []).append(lambda: qk_chain(0, 1, 0))
                inserts.setdefault(12, []).append(lambda: qk_chain(1, 1, 0))
                inserts.setdefault(13, []).append(lambda: qk_chain(0, 1, 1))
                inserts.setdefault(17, []).append(lambda: qk_chain(1, 1, 1))
                inserts.setdefault(21, []).append(lambda: qk_chain(1, 1, 2))
                inserts.setdefault(25, []).append(lambda: qk_chain(1, 1, 3))
                inserts.setdefault(28, []).append(lambda: qk_chain(0, 0, 2))
                inserts.setdefault(30, []).append(lambda: qk_chain(0, 0, 3))
                inserts.setdefault(44, []).append(lambda: qk_chain(0, 1, 2))
                inserts.setdefault(46, []).append(lambda: qk_chain(0, 1, 3))


                # ---- phase 2: attention, software-pipelined emission ----
                ctxn = [
                    qk.tile([128, N], MMT, tag=f"ctxn{m}", name=f"ctxn{m}")
                    for m in range(2)
                ]

                blocks = [(ih, hp) for ih in range(2) for hp in range(2)]
                units = [
                    (b_idx, ih, hp, jt)
                    for b_idx, (ih, hp) in enumerate(blocks)
                    for jt in range(16)
                ]
                ctx_ps_of = {}
                unit_e = {}

                def emit_s_exp(u):
                    b_idx, ih, hp, jt = u
                    e_sb = [
                        epool.tile([128, 1024], MMT, tag="e", name="esb")
                        for _ in range(2)
                    ]
                    s_ps2 = [
                        psum_b.tile([128, 1024], F32, tag="bank", name="sps")
                        for _ in range(2)
                    ]
                    for h2 in range(2):
                        for it in range(2):
                            nc.tensor.matmul(
                                s_ps2[h2][:, it * 512 : (it + 1) * 512],
                                _mm_ap(
                                    kt_sb[hp][
                                        h2 * 64 : (h2 + 1) * 64,
                                        jt * 128 : (jt + 1) * 128,
                                    ]
                                ),
                                _mm_ap(
                                    qt_sb[hp][
                                        h2 * 64 : (h2 + 1) * 64,
                                        ih * 1024 + it * 512 : ih * 1024
                                        + (it + 1) * 512,
                                    ]
                                ),
                                start=True,
                                stop=True,
                            )
                    for h2 in range(2):
                        nc.scalar.activation(
                            out=e_sb[h2],
                            in_=s_ps2[h2],
                            func=mybir.ActivationFunctionType.Exp,
                            bias=mb_sb[:, jt : jt + 1],
                            scale=0.125,
                        )
                    unit_e[u] = e_sb

                def emit_ctx(u):
                    b_idx, ih, hp, jt = u
                    if b_idx not in ctx_ps_of:
                        ctx_ps_of[b_idx] = [
                            psum_c.tile([HD + 1, 1024], F32, tag="ctx", name="ctxps")
                            for _ in range(2)
                        ]
                    ctx_ps = ctx_ps_of[b_idx]
                    e_sb = unit_e.pop(u)
                    for h2 in range(2):
                        for it in range(2):
                            nc.tensor.matmul(
                                ctx_ps[h2][:, it * 512 : (it + 1) * 512],
                                _mm_ap(v_sb[:, jt, 2 * hp + h2, :]),
                                _mm_ap(e_sb[h2][:, it * 512 : (it + 1) * 512]),
                                start=(jt == 0),
                                stop=(jt == 15),
                                skip_group_check=True,
                            )

                def emit_norm(b_idx):
                    ih, hp = blocks[b_idx]
                    ctx_ps = ctx_ps_of[b_idx]
                    for h2 in (1, 0):
                        r_sb = rpool.tile([65, 1024], F32R, tag="r", name="rsb")
                        nc.vector.reciprocal(
                            out=r_sb[64:65, :], in_=ctx_ps[h2][64:65, :]
                        )
                        for it in range(2):
                            rp = psum_b.tile([64, 512], F32, tag="bank", name="rp")
                            nc.tensor.matmul(
                                rp,
                                _mm_ap(ones[64:65, 0:64]),
                                _mm_ap(r_sb[64:65, it * 512 : (it + 1) * 512]),
                                start=True,
                                stop=True,
                                tile_position=(64, 0),
                            )
                            nc.vector.tensor_copy(
                                r_sb[0:64, it * 512 : (it + 1) * 512], rp
                            )
                        if h2 == 0:
                            nc.vector.tensor_tensor(
                                out=ctxn[hp][0:64, ih * 1024 : (ih + 1) * 1024],
                                in0=ctx_ps[0][0:64, :],
                                in1=r_sb[0:64, :],
                                op=mybir.AluOpType.mult,
                            )
                        else:
                            tmp = big.tile([64, 1024], MMT, tag="big", name="tmp")
                            nc.vector.tensor_tensor(
                                out=tmp,
                                in0=ctx_ps[1][0:64, :],
                                in1=r_sb[0:64, :],
                                op=mybir.AluOpType.mult,
                            )
                            # partition shift 0-63 -> 64-127 via SBUF->SBUF DMA
                            nc.sync.dma_start(
                                out=ctxn[hp][64:128, ih * 1024 : (ih + 1) * 1024],
                                in_=tmp,
                            )

                def emit_outproj(ih, mo_list=None, copy_eng=None):
                    for mo in (range(8) if mo_list is None else mo_list):
                        ps = psum_b.tile([128, 1024], F32, tag="bank", name="ops")
                        for nt2 in range(2):
                            nt = 2 * ih + nt2
                            for kt in range(2):
                                nc.tensor.matmul(
                                    ps[:, nt2 * 512 : (nt2 + 1) * 512],
                                    _mm_ap(wo_sb[:, kt, mo * 128 : (mo + 1) * 128]),
                                    _mm_ap(ctxn[kt][:, nt * 512 : (nt + 1) * 512]),
                                    start=(kt == 0),
                                    stop=(kt == 1),
                                )
                        ob = big.tile([128, 1024], F32, tag="big", name="ob")
                        (copy_eng or nc.vector.tensor_copy)(ob, ps)
                        nc.sync.dma_start(
                            out=outp[
                                mo * 128 : (mo + 1) * 128,
                                ih * 1024 : (ih + 1) * 1024,
                            ],
                            in_=ob,
                        )

                extras = {}
                for b_idx, (ih, hp) in enumerate(blocks):
                    last = 16 * (b_idx + 1) - 1
                    extras.setdefault(last + 1, []).append(
                        lambda b=b_idx: emit_norm(b)
                    )
                    if hp == 1:
                        for j, mo in enumerate(range(8)):
                            # tail (ih==1): both ScalarE and DVE are idle, so
                            # alternate the psum->sbuf copies across them to
                            # halve the final drain
                            ce = (
                                (nc.scalar.copy if mo % 2 == 0 else None)
                                if ih == 1
                                else None
                            )
                            extras.setdefault(last + 3 + j, []).append(
                                lambda i=ih, m=mo, c=ce: emit_outproj(i, [m], c)
                            )

                trailing = []
                for i, u in enumerate(units):
                    emit_s_exp(u)
                    if i > 0:
                        emit_ctx(units[i - 1])
                    for fn in inserts.get(i, []):
                        fn()
                    for fn in extras.get(i, []):
                        if i == len(units) - 1:
                            trailing.append(fn)
                        else:
                            fn()
                emit_ctx(units[-1])
                for i in sorted(extras):
                    if i >= len(units):
                        trailing.extend(extras[i])
                for fn in trailing:
                    fn()

    nc.finalize()
    return nc


_NC_CACHE = None


def _get_program():
    global _NC_CACHE
    if _NC_CACHE is None:
        _NC_CACHE = build_program()
    return _NC_CACHE


def make_in_maps(X, mask, Wq, bq, Wk, bk, Wv, bv, Wo, bo):
    import ml_dtypes

    BF_NP = ml_dtypes.bfloat16
    X = np.asarray(X, dtype=np.float32)
    mask = np.asarray(mask, dtype=np.float32)
    in_maps = []
    xts = [np.ascontiguousarray(X[b].T).astype(BF_NP) for b in range(B)]
    mbs = [
        np.ascontiguousarray((-1e6 * (1.0 - mask[b])).reshape(16, 128).T)
        for b in range(B)
    ]
    for c in range(8):
        b, g = c // HG, c % HG
        sl = slice(g * DG, (g + 1) * DG)
        wq_s = np.ascontiguousarray(
            np.asarray(Wq[:, sl]).reshape(8, 128, DG).transpose(1, 0, 2)
        )
        wk_s = np.ascontiguousarray(
            np.asarray(Wk[:, sl]).reshape(8, 128, DG).transpose(1, 0, 2)
        )
        wv_s = np.ascontiguousarray(
            np.asarray(Wv[:, sl]).reshape(8, 128, DG).transpose(1, 0, 2)
        )
        bq_s = np.ascontiguousarray(np.asarray(bq[sl]).reshape(2, 128).T)
        bk_s = np.ascontiguousarray(np.asarray(bk[sl]).reshape(2, 128).T)
        bv_s = np.ascontiguousarray(np.asarray(bv[sl]).reshape(1, DG))
        # Wo rows for this group, pair-packed: [64*h2+p, kt, o] = Wo[g*256+(2kt+h2)*64+p, o]
        wo_s = np.ascontiguousarray(
            np.asarray(Wo[sl, :]).reshape(2, 2, 64, D).transpose(1, 2, 0, 3)
            .reshape(128, 2, D)
        )
        in_maps.append(
            {
                "xt": xts[b],
                "onesin": np.ones((128, 128), dtype=np.float32),
                "onesb": np.ones((128, 64), dtype=BF_NP),
                "wq": wq_s.astype(BF_NP),
                "wk": wk_s.astype(BF_NP),
                "wv": wv_s.astype(BF_NP),
                "bq": bq_s.astype(np.float32),
                "bk": bk_s.astype(np.float32),
                "bvr": bv_s.astype(np.float32),
                "wo": wo_s.astype(BF_NP),
                "mb": mbs[b].astype(np.float32),
            }
        )
    return in_maps


def gather_output(results, bo):
    out = np.zeros((B, N, D), dtype=np.float32)
    for c in range(8):
        out[c // HG] += results[c]["outp"].T
    out += np.asarray(bo, dtype=np.float32)
    return out


def kernel(**inputs):
    from concourse import bass_utils

    nc = _get_program()
    in_maps = make_in_maps(**inputs)
    res = bass_utils.run_bass_kernel_spmd(nc, in_maps, core_ids=list(range(8)))
    return gather_output(res.results, inputs["bo"])



# revision 18
# speedup vs baseline: 2.8600x; 2.8600x over previous
"""Multi-head attention (B=2, N=2048, D=1024, H=16, HD=64) on 8 trn2 NeuronCores.

Sharding: data-parallel over batch (2) x tensor-parallel over head groups (4).
Core c handles batch b=c//4, heads 4*(c%4)..4*(c%4)+3. Each core computes
Q/K/V projections for its head slice, attention, and a partial output
projection (its heads' rows of Wo); the host sums the 4 partials per batch
and adds bo.

Device layout strategy: everything lives feature-on-partitions ("transposed")
so no on-device transposes are needed:
  - host passes X[b].T; Q^T/K^T computed as (W^T X^T) with W as stationary.
  - V computed in native [token, d] layout (X^T tiles as stationary).
  - scores computed as S^T[j, i] (key j on partitions) so the mask bias is a
    per-partition scalar and softmax normalization is deferred:
    E^T = exp(S/8 + maskbias) via one ScalarE activation (PSUM->SBUF).
  - ctx^T[d, i] = sum_j V_aug[j, d] E^T[j, i]; V_aug has a ones column so the
    softmax denominator rides along as ctx row 64.
  - normalization multiplies ctx^T by 1/denom broadcast via a tiny PE matmul.
  - out^T = Wo^T ctx^T accumulated over head pairs; host transposes back.
"""

import sys

if "/opt/trn_rl_repo" not in sys.path:
    sys.path.insert(0, "/opt/trn_rl_repo")

import numpy as np

import concourse.bacc as bacc
import concourse.mybir as mybir
import concourse.tile as tile

B, N, D = 2, 2048, 1024
H, HD = 16, 64
HG = 4  # head groups (tensor parallel)
HPG = H // HG  # heads per group = 4
DG = HPG * HD  # feature slice per group = 256

F32 = mybir.dt.float32
# Matmul datapath dtype: bf16 streams 1 row/cycle at any size, halves HBM
# and SBUF traffic, and doubles DVE write throughput. Accumulation stays
# fp32 in PSUM. float32r is kept for the tiny norm-broadcast/bias matmuls
# whose operands must stay full precision.
MMT = mybir.dt.bfloat16
F32R = mybir.dt.float32r


def _mm_ap(ap):
    return ap


def build_program(loop_iters: int = 1):
    nc = bacc.Bacc("TRN2", target_bir_lowering=False)

    xt = nc.dram_tensor("xt", [D, N], MMT, kind="ExternalInput")
    wq = nc.dram_tensor("wq", [128, 8, DG], MMT, kind="ExternalInput")
    wk = nc.dram_tensor("wk", [128, 8, DG], MMT, kind="ExternalInput")
    wv = nc.dram_tensor("wv", [128, 8, DG], MMT, kind="ExternalInput")
    bq = nc.dram_tensor("bq", [128, 2], F32, kind="ExternalInput")
    bk = nc.dram_tensor("bk", [128, 2], F32, kind="ExternalInput")
    bvr = nc.dram_tensor("bvr", [1, DG], F32R, kind="ExternalInput")
    wo = nc.dram_tensor("wo", [128, 2, D], MMT, kind="ExternalInput")
    mb = nc.dram_tensor("mb", [128, 16], F32, kind="ExternalInput")
    onesin = nc.dram_tensor("onesin", [128, 128], F32R, kind="ExternalInput")
    onesb = nc.dram_tensor("onesb", [128, 64], MMT, kind="ExternalInput")
    outp = nc.dram_tensor("outp", [D, N], MMT, kind="ExternalOutput")

    with tile.TileContext(nc) as tc, nc.allow_low_precision(
        reason="fp32r matmul datapath; accumulation stays fp32 in PSUM"
    ):
        import contextlib

        ctx = contextlib.ExitStack()
        with ctx:
            const = ctx.enter_context(tc.tile_pool(name="const", bufs=1))
            big = ctx.enter_context(tc.tile_pool(name="big", bufs=5))
            xtcp = ctx.enter_context(tc.tile_pool(name="xtcp", bufs=4))
            qk = ctx.enter_context(tc.tile_pool(name="qk", bufs=1))
            epool = ctx.enter_context(tc.tile_pool(name="epool", bufs=5))
            rpool = ctx.enter_context(tc.tile_pool(name="rpool", bufs=2))
            psum_b = ctx.enter_context(
                tc.tile_pool(name="psum_b", bufs=2, space="PSUM")
            )
            psum_c = ctx.enter_context(
                tc.tile_pool(name="psum_c", bufs=2, space="PSUM")
            )

            loop_cm = (
                tc.For_i(0, loop_iters, 1)
                if loop_iters > 1
                else contextlib.nullcontext()
            )
            with loop_cm:
                # ---- phase 1 loads first: wq + X chunk 0 gate the first
                # matmul chain, so issue them before the small const DMAs.
                wq_sb = big.tile([128, 8, DG], MMT, tag="big")
                nc.sync.dma_start(out=wq_sb[:, :, 0:128], in_=wq[:, :, 0:128])
                nc.sync.dma_start(out=wq_sb[:, :, 128:DG], in_=wq[:, :, 128:DG])
                wk_sb = big.tile([128, 8, DG], MMT, tag="big")
                nc.gpsimd.dma_start(out=wk_sb, in_=wk[:, :, :])
                wv_sb = big.tile([128, 8, DG], MMT, tag="big")
                nc.gpsimd.dma_start(out=wv_sb, in_=wv[:, :, :])

                # ---- constants ----
                ones = const.tile([128, 128], F32R, tag="ones")
                nc.sync.dma_start(out=ones, in_=onesin[:, :])
                bq_sb = const.tile([128, 2], F32, tag="bq")
                nc.sync.dma_start(out=bq_sb, in_=bq[:, :])
                bk_sb = const.tile([128, 2], F32, tag="bk")
                nc.sync.dma_start(out=bk_sb, in_=bk[:, :])
                bvr_sb = const.tile([1, DG], F32R, tag="bvr")
                nc.sync.dma_start(out=bvr_sb, in_=bvr[:, :])
                mb_sb = const.tile([128, 16], F32, tag="mb")
                nc.sync.dma_start(out=mb_sb, in_=mb[:, :])
                wo_sb = const.tile([128, 2, D], MMT, tag="wo")

                # bv broadcast to all 128 partitions via PE
                bv_ps = psum_b.tile([128, DG], F32, tag="bank")
                nc.tensor.matmul(
                    bv_ps, _mm_ap(ones[0:1, 0:128]), _mm_ap(bvr_sb[0:1, :]),
                    start=True, stop=True,
                )
                bv_bc = const.tile([128, DG], F32, tag="bvbc")
                nc.vector.tensor_copy(bv_bc, bv_ps)
                # X^T in 4 column-chunk tiles [128, kt=8, 512]; one DMA each,
                # chunk-major so early Q/K/V chains start before the full X
                # load completes. xtc[c][kt] view = features kt*128.., tokens
                # c*512..
                xtc_t = []
                for c in range(4):
                    t = xtcp.tile([128, 8, 512], MMT, tag="xtc", name="xtc")
                    eng = nc.sync if c < 2 else nc.gpsimd
                    nparts = 4 if c == 0 else 2
                    for h in range(nparts):
                        kpp = 8 // nparts
                        eng.dma_start(
                            out=t[:, h * kpp : (h + 1) * kpp, :],
                            in_=xt[
                                h * kpp * 128 : (h + 1) * kpp * 128,
                                c * 512 : (c + 1) * 512,
                            ].rearrange("(kt p) col -> p kt col", p=128),
                        )
                    xtc_t.append(t)
                xtc = [[xtc_t[c][:, kt, :] for kt in range(8)] for c in range(4)]

                qt_sb = [qk.tile([128, N], MMT, tag=f"qt{m}", name=f"qt{m}") for m in range(2)]
                kt_sb = [qk.tile([128, N], MMT, tag=f"kt{m}", name=f"kt{m}") for m in range(2)]
                # V with ones column appended per head: [128, jt, head, 65]
                v_sb = qk.tile([128, 16, HPG, HD + 1], MMT, tag="v")
                nc.gpsimd.dma_start(
                    out=v_sb[:, :, :, HD : HD + 1], in_=onesb[:, :]
                )

                def qk_chain(proj, mt, nt):
                    w_sb, bias_sb, dst = (
                        (wq_sb, bq_sb, qt_sb) if proj == 0 else (wk_sb, bk_sb, kt_sb)
                    )
                    ps = psum_b.tile([128, 512], F32, tag="bank", name="qkps")
                    for kt in range(8):
                        nc.tensor.matmul(
                            ps,
                            _mm_ap(w_sb[:, kt, mt * 128 : (mt + 1) * 128]),
                            _mm_ap(xtc[nt][kt]),
                            start=(kt == 0),
                            stop=(kt == 7),
                        )
                    nc.vector.tensor_scalar_add(
                        dst[mt][:, nt * 512 : (nt + 1) * 512],
                        ps,
                        bias_sb[:, mt : mt + 1],
                    )

                def v_chain(mt, hp):
                    # half chain: V columns for head pair hp of j-tile mt
                    ps = psum_b.tile([128, 128], F32, tag="bank", name="vps")
                    for kt in range(8):
                        nc.tensor.matmul(
                            ps,
                            _mm_ap(
                                xtc[mt // 4][kt][
                                    :, (mt % 4) * 128 : (mt % 4 + 1) * 128
                                ]
                            ),
                            _mm_ap(wv_sb[:, kt, hp * 128 : (hp + 1) * 128]),
                            start=(kt == 0),
                            stop=(kt == 7),
                        )
                    nc.vector.tensor_tensor(
                        out=v_sb[:, mt, 2 * hp : 2 * hp + 2, 0:HD],
                        in0=ps.rearrange("p (h d) -> p h d", h=2),
                        in1=bv_bc[:, hp * 128 : (hp + 1) * 128].rearrange(
                            "p (h d) -> p h d", h=2
                        ),
                        op=mybir.AluOpType.add,
                    )

                # chains needed before the unit stream starts (block 0 =
                # (ih0, hp0): needs qt[0] nt 0-1, kt[0] nt 0, V pair 0 of jt 0)
                for fn in (
                    lambda: qk_chain(0, 0, 0),
                    lambda: qk_chain(1, 0, 0),
                    lambda: qk_chain(0, 0, 1),
                    lambda: v_chain(0, 0),
                ):
                    fn()

                # remaining chains, spread through the unit stream (key =
                # iteration index at whose END the chain is emitted; each must
                # precede its first consumer unit)
                inserts = {}
                for i in range(1, 16):
                    inserts.setdefault(i, []).append(lambda m=i: v_chain(m, 0))
                for i in range(16, 32):
                    inserts.setdefault(i, []).append(
                        lambda m=i - 16: v_chain(m, 1)
                    )
                inserts.setdefault(3, []).append(lambda: qk_chain(1, 0, 1))
                inserts.setdefault(7, []).append(lambda: qk_chain(1, 0, 2))
                inserts.setdefault(11, []).append(lambda: qk_chain(1, 0, 3))
                inserts.setdefault(13, []).append(lambda: qk_chain(0, 0, 2))
                inserts.setdefault(14, []).append(lambda: qk_chain(0, 0, 3))
                inserts.setdefault(20, []).append(
                    lambda: nc.sync.dma_start(out=wo_sb, in_=wo[:, :, :])
                )
                inserts.setdefault(26, []).append(lambda: qk_chain(0, 1, 0))
                inserts.setdefault(28, []).append(lambda: qk_chain(0, 1, 1))
                inserts.setdefault(30, []).append(lambda: qk_chain(1, 1, 0))
                inserts.setdefault(33, []).append(lambda: qk_chain(1, 1, 1))
                inserts.setdefault(37, []).append(lambda: qk_chain(1, 1, 2))
                inserts.setdefault(41, []).append(lambda: qk_chain(1, 1, 3))
                inserts.setdefault(44, []).append(lambda: qk_chain(0, 1, 2))
                inserts.setdefault(46, []).append(lambda: qk_chain(0, 1, 3))


                # ---- phase 2: attention, software-pipelined emission ----
                ctxn = [
                    qk.tile([128, N], MMT, tag=f"ctxn{m}", name=f"ctxn{m}")
                    for m in range(2)
                ]

                # hp-major: hp=1 inputs (qt/kt/v chains) aren't needed until
                # unit 32, so phase-1 chain work spreads over two blocks
                blocks = [(ih, hp) for hp in range(2) for ih in range(2)]
                units = [
                    (b_idx, ih, hp, jt)
                    for b_idx, (ih, hp) in enumerate(blocks)
                    for jt in range(16)
                ]
                ctx_ps_of = {}
                unit_e = {}

                def emit_s_exp(u):
                    b_idx, ih, hp, jt = u
                    e_sb = [
                        epool.tile([128, 1024], MMT, tag="e", name="esb")
                        for _ in range(2)
                    ]
                    s_ps2 = [
                        psum_b.tile([128, 1024], F32, tag="bank", name="sps")
                        for _ in range(2)
                    ]
                    for h2 in range(2):
                        for it in range(2):
                            nc.tensor.matmul(
                                s_ps2[h2][:, it * 512 : (it + 1) * 512],
                                _mm_ap(
                                    kt_sb[hp][
                                        h2 * 64 : (h2 + 1) * 64,
                                        jt * 128 : (jt + 1) * 128,
                                    ]
                                ),
                                _mm_ap(
                                    qt_sb[hp][
                                        h2 * 64 : (h2 + 1) * 64,
                                        ih * 1024 + it * 512 : ih * 1024
                                        + (it + 1) * 512,
                                    ]
                                ),
                                start=True,
                                stop=True,
                            )
                    for h2 in range(2):
                        nc.scalar.activation(
                            out=e_sb[h2],
                            in_=s_ps2[h2],
                            func=mybir.ActivationFunctionType.Exp,
                            bias=mb_sb[:, jt : jt + 1],
                            scale=0.125,
                        )
                    unit_e[u] = e_sb

                def emit_ctx(u):
                    b_idx, ih, hp, jt = u
                    if b_idx not in ctx_ps_of:
                        ctx_ps_of[b_idx] = [
                            psum_c.tile([HD + 1, 1024], F32, tag="ctx", name="ctxps")
                            for _ in range(2)
                        ]
                    ctx_ps = ctx_ps_of[b_idx]
                    e_sb = unit_e.pop(u)
                    for h2 in range(2):
                        for it in range(2):
                            nc.tensor.matmul(
                                ctx_ps[h2][:, it * 512 : (it + 1) * 512],
                                _mm_ap(v_sb[:, jt, 2 * hp + h2, :]),
                                _mm_ap(e_sb[h2][:, it * 512 : (it + 1) * 512]),
                                start=(jt == 0),
                                stop=(jt == 15),
                                skip_group_check=True,
                            )

                def emit_norm(b_idx):
                    ih, hp = blocks[b_idx]
                    ctx_ps = ctx_ps_of[b_idx]
                    for h2 in (1, 0):
                        r_sb = rpool.tile([65, 1024], F32R, tag="r", name="rsb")
                        nc.vector.reciprocal(
                            out=r_sb[64:65, :], in_=ctx_ps[h2][64:65, :]
                        )
                        for it in range(2):
                            rp = psum_b.tile([64, 512], F32, tag="bank", name="rp")
                            nc.tensor.matmul(
                                rp,
                                _mm_ap(ones[64:65, 0:64]),
                                _mm_ap(r_sb[64:65, it * 512 : (it + 1) * 512]),
                                start=True,
                                stop=True,
                                tile_position=(64, 0),
                            )
                            nc.vector.tensor_copy(
                                r_sb[0:64, it * 512 : (it + 1) * 512], rp
                            )
                        if h2 == 0:
                            nc.vector.tensor_tensor(
                                out=ctxn[hp][0:64, ih * 1024 : (ih + 1) * 1024],
                                in0=ctx_ps[0][0:64, :],
                                in1=r_sb[0:64, :],
                                op=mybir.AluOpType.mult,
                            )
                        else:
                            tmp = big.tile([64, 1024], MMT, tag="big", name="tmp")
                            nc.vector.tensor_tensor(
                                out=tmp,
                                in0=ctx_ps[1][0:64, :],
                                in1=r_sb[0:64, :],
                                op=mybir.AluOpType.mult,
                            )
                            # partition shift 0-63 -> 64-127 via SBUF->SBUF DMA
                            nc.sync.dma_start(
                                out=ctxn[hp][64:128, ih * 1024 : (ih + 1) * 1024],
                                in_=tmp,
                            )

                def emit_outproj(ih, mo_list=None, copy_eng=None):
                    for mo in (range(8) if mo_list is None else mo_list):
                        ps = psum_b.tile([128, 1024], F32, tag="bank", name="ops")
                        for nt2 in range(2):
                            nt = 2 * ih + nt2
                            for kt in range(2):
                                nc.tensor.matmul(
                                    ps[:, nt2 * 512 : (nt2 + 1) * 512],
                                    _mm_ap(wo_sb[:, kt, mo * 128 : (mo + 1) * 128]),
                                    _mm_ap(ctxn[kt][:, nt * 512 : (nt + 1) * 512]),
                                    start=(kt == 0),
                                    stop=(kt == 1),
                                )
                        ob = big.tile([128, 1024], MMT, tag="big", name="ob")
                        (copy_eng or nc.vector.tensor_copy)(ob, ps)
                        nc.sync.dma_start(
                            out=outp[
                                mo * 128 : (mo + 1) * 128,
                                ih * 1024 : (ih + 1) * 1024,
                            ],
                            in_=ob,
                        )

                extras = {}
                for b_idx in range(4):
                    extras.setdefault(16 * (b_idx + 1), []).append(
                        lambda b=b_idx: emit_norm(b)
                    )
                # outproj(ih=0) needs ctxn[0][ih0] (norm b0) + ctxn[1][ih0]
                # (norm b2, emitted at 48): spread through block 3.
                for j, mo in enumerate(range(8)):
                    extras.setdefault(49 + j, []).append(
                        lambda m=mo: emit_outproj(0, [m])
                    )
                # outproj(ih=1) trails the last block; both ScalarE and DVE
                # are idle there, so alternate the psum->sbuf copies.
                for j, mo in enumerate(range(8)):
                    ce = nc.scalar.copy if mo % 2 == 0 else None
                    extras.setdefault(65 + j, []).append(
                        lambda m=mo, c=ce: emit_outproj(1, [m], c)
                    )

                trailing = []
                for i, u in enumerate(units):
                    emit_s_exp(u)
                    if i > 0:
                        emit_ctx(units[i - 1])
                    for fn in inserts.get(i, []):
                        fn()
                    for fn in extras.get(i, []):
                        if i == len(units) - 1:
                            trailing.append(fn)
                        else:
                            fn()
                emit_ctx(units[-1])
                for i in sorted(extras):
                    if i >= len(units):
                        trailing.extend(extras[i])
                for fn in trailing:
                    fn()

    nc.finalize()
    return nc


_NC_CACHE = None


def _get_program():
    global _NC_CACHE
    if _NC_CACHE is None:
        _NC_CACHE = build_program()
    return _NC_CACHE


def make_in_maps(X, mask, Wq, bq, Wk, bk, Wv, bv, Wo, bo):
    import ml_dtypes

    BF_NP = ml_dtypes.bfloat16
    X = np.asarray(X, dtype=np.float32)
    mask = np.asarray(mask, dtype=np.float32)
    in_maps = []
    xts = [np.ascontiguousarray(X[b].T).astype(BF_NP) for b in range(B)]
    mbs = [
        np.ascontiguousarray((-1e6 * (1.0 - mask[b])).reshape(16, 128).T)
        for b in range(B)
    ]
    for c in range(8):
        b, g = c // HG, c % HG
        sl = slice(g * DG, (g + 1) * DG)
        wq_s = np.ascontiguousarray(
            np.asarray(Wq[:, sl]).reshape(8, 128, DG).transpose(1, 0, 2)
        )
        wk_s = np.ascontiguousarray(
            np.asarray(Wk[:, sl]).reshape(8, 128, DG).transpose(1, 0, 2)
        )
        wv_s = np.ascontiguousarray(
            np.asarray(Wv[:, sl]).reshape(8, 128, DG).transpose(1, 0, 2)
        )
        bq_s = np.ascontiguousarray(np.asarray(bq[sl]).reshape(2, 128).T)
        bk_s = np.ascontiguousarray(np.asarray(bk[sl]).reshape(2, 128).T)
        bv_s = np.ascontiguousarray(np.asarray(bv[sl]).reshape(1, DG))
        # Wo rows for this group, pair-packed: [64*h2+p, kt, o] = Wo[g*256+(2kt+h2)*64+p, o]
        wo_s = np.ascontiguousarray(
            np.asarray(Wo[sl, :]).reshape(2, 2, 64, D).transpose(1, 2, 0, 3)
            .reshape(128, 2, D)
        )
        in_maps.append(
            {
                "xt": xts[b],
                "onesin": np.ones((128, 128), dtype=np.float32),
                "onesb": np.ones((128, 64), dtype=BF_NP),
                "wq": wq_s.astype(BF_NP),
                "wk": wk_s.astype(BF_NP),
                "wv": wv_s.astype(BF_NP),
                "bq": bq_s.astype(np.float32),
                "bk": bk_s.astype(np.float32),
                "bvr": bv_s.astype(np.float32),
                "wo": wo_s.astype(BF_NP),
                "mb": mbs[b].astype(np.float32),
            }
        )
    return in_maps


def gather_output(results, bo):
    out = np.zeros((B, N, D), dtype=np.float32)
    for c in range(8):
        out[c // HG] += np.asarray(results[c]["outp"], dtype=np.float32).T
    out += np.asarray(bo, dtype=np.float32)
    return out


def kernel(**inputs):
    from concourse import bass_utils

    nc = _get_program()
    in_maps = make_in_maps(**inputs)
    res = bass_utils.run_bass_kernel_spmd(nc, in_maps, core_ids=list(range(8)))
    return gather_output(res.results, inputs["bo"])

